# revision 5
# baseline (speedup 1.0000x reference)
"""Trainium2 Bass kernel for nn_Cov_2 (retrieval_knn pairwise-L2 / masked column mean).

Strategy (8 NeuronCores, SPMD):
  - Host compacts: Q' = seq rows with qvs_idx!=0 (queries), S' = seq rows with
    sum_idx!=0 (the masked key set).  Rows/columns outside the masks contribute
    closed-form terms (sqrt(q2_i) / sqrt(s2_j) / 0) that are folded in as
    scalar corrections, so the device only computes the Nq x Ns dense block.
  - Q' rows are sharded across the 8 cores; S'^T, s2 are replicated.
  - Per core: dist' = sqrt(relu(-2*Q S^T + s2 + q2)) via TensorE (bf16) with the
    s2 row folded in as a K=1 augmented matmul and q2 as a per-partition DVE
    scalar; ACT does sqrt with free accumulation of row sums.
  - Only the scalar partial mean `norm` is AllReduced across cores; the final
    normalize happens on-device, host just scatters rows back.
"""

import numpy as np
import ml_dtypes

import concourse.bass as bass
import concourse.mybir as mybir
import concourse.tile as tile
from concourse import bacc
from concourse.bass_utils import run_bass_kernel_spmd

F32 = mybir.dt.float32
BF16 = mybir.dt.bfloat16
BF16_NP = ml_dtypes.bfloat16

N_CORES = 8
D = 512
K_TILES = 4  # D // 128
NEG_BIG = -1.0e6

_cache = {}


def _build_program(R, NS_PAD, sc):
    """Build the SPMD Bass program for one core (R query rows, NS_PAD key cols).

    sc: dict of python-float constants baked as immediates:
        inv_count, n_minus_ns, n_minus_nq, inv_n2, w, b
    """
    M_TILES = R // 128
    N_TILES = NS_PAD // 512
    S2F_COLS = NS_PAD // 128
    AF = mybir.ActivationFunctionType
    OP = mybir.AluOpType

    nc = bacc.Bacc("TRN2", target_bir_lowering=False, debug=False,
                   num_devices=N_CORES)

    qt = nc.dram_tensor("qt", [K_TILES, 128, R], BF16, kind="ExternalInput").ap()
    st = nc.dram_tensor("st", [K_TILES, 128, NS_PAD], BF16, kind="ExternalInput").ap()
    s2aug = nc.dram_tensor("s2aug", [1, NS_PAD], BF16, kind="ExternalInput").ap()
    q2b = nc.dram_tensor("q2b", [128, M_TILES], F32, kind="ExternalInput").ap()
    q2f = nc.dram_tensor("q2f", [128, M_TILES], F32, kind="ExternalInput").ap()
    s2f = nc.dram_tensor("s2f", [128, S2F_COLS], F32, kind="ExternalInput").ap()
    out = nc.dram_tensor("out", [R, 1], F32, kind="ExternalOutput").ap()
    outc = nc.dram_tensor("outc", [1, 1], F32, kind="ExternalOutput").ap()

    with tile.TileContext(nc, num_cores=N_CORES) as tc:
        with (
            tc.tile_pool(name="persist", bufs=1) as persist,
            tc.tile_pool(name="work", bufs=3) as work,
            tc.tile_pool(name="tiny", bufs=2) as tiny,
            tc.tile_pool(name="mm_psum", bufs=4, space="PSUM") as mm_psum,
            tc.tile_pool(name="sc_psum", bufs=1, space="PSUM") as sc_psum,
            tc.tile_pool(name="dram", bufs=1, space="DRAM") as dram,
        ):
            def ptile(shape, dtype, name):
                return persist.tile(shape, dtype, name=name, tag=name)

            # ---- persistent tiles ----
            ones_bf = ptile([1, 128], BF16, name="ones_bf")
            nc.vector.memset(ones_bf[:], 1.0)
            ones_bcast = ptile([1, 128], F32, name="ones_bcast")
            nc.vector.memset(ones_bcast[:], 1.0)
            ones_red = ptile([128, 1], F32, name="ones_red")
            nc.vector.memset(ones_red[:], 1.0)

            s2aug_sb = ptile([1, NS_PAD], BF16, name="s2aug_sb")
            nc.sync.dma_start(s2aug_sb[:], s2aug[:, :])
            q2b_sb = ptile([128, M_TILES], F32, name="q2b_sb")
            nc.sync.dma_start(q2b_sb[:], q2b[:, :])
            q2f_sb = ptile([128, M_TILES], F32, name="q2f_sb")
            nc.sync.dma_start(q2f_sb[:], q2f[:, :])
            s2f_sb = ptile([128, S2F_COLS], F32, name="s2f_sb")
            nc.sync.dma_start(s2f_sb[:], s2f[:, :])

            qt_sb = []
            for k in range(K_TILES):
                t = ptile([128, R], BF16, name=f"qt_sb{k}")
                nc.sync.dma_start(t[:], qt[k, :, :])
                qt_sb.append(t)
            st_sb = []
            for k in range(K_TILES):
                t = ptile([128, NS_PAD], BF16, name=f"st_sb{k}")
                # chunked loads so matmuls can start before the whole row is in
                for n in range(N_TILES):
                    nc.sync.dma_start(t[:, n * 512:(n + 1) * 512],
                                      st[k, :, n * 512:(n + 1) * 512])
                st_sb.append(t)

            rsum = ptile([128, M_TILES], F32, name="rsum")
            accs = [ptile([128, N_TILES], F32, name=f"acc{m}")
                    for m in range(M_TILES)]

            # ---- main distance block ----
            for m in range(M_TILES):
                ms = slice(m * 128, (m + 1) * 128)
                for n in range(N_TILES):
                    ns = slice(n * 512, (n + 1) * 512)
                    ps = mm_psum.tile([128, 512], F32, tag="mm")
                    for k in range(K_TILES):
                        nc.tensor.matmul(ps[:], qt_sb[k][:, ms], st_sb[k][:, ns],
                                         start=(k == 0), stop=False)
                    nc.tensor.matmul(ps[:], ones_bf[:, :], s2aug_sb[:, ns],
                                     start=False, stop=True)
                    # u = max(g' + q2_i, 0)   (q2 per-partition, relu fused)
                    u = work.tile([128, 512], BF16, tag="u")
                    nc.vector.tensor_scalar(u[:], ps[:], q2b_sb[:, m:m + 1], 0.0,
                                            OP.add, OP.max)
                    # dist = sqrt(u); row-sum comes free via accum_out
                    dist = work.tile([128, 512], BF16, tag="dist")
                    nc.scalar.activation(dist[:], u[:], AF.Sqrt,
                                         accum_out=accs[m][:, n:n + 1])
                nc.vector.reduce_sum(rsum[:, m:m + 1], accs[m][:, 0:N_TILES],
                                     axis=mybir.AxisListType.X)

            # ---- scalar reductions: Tq, Ts, total row-sum ----
            sq_q = ptile([128, M_TILES], F32, name="sq_q")
            tq_acc = ptile([128, 1], F32, name="tq_acc")
            nc.scalar.activation(sq_q[:], q2f_sb[:], AF.Sqrt, accum_out=tq_acc[:])
            sq_s = ptile([128, S2F_COLS], F32, name="sq_s")
            ts_acc = ptile([128, 1], F32, name="ts_acc")
            nc.scalar.activation(sq_s[:], s2f_sb[:], AF.Sqrt, accum_out=ts_acc[:])

            rs_tot = ptile([128, 1], F32, name="rs_tot")
            nc.vector.reduce_sum(rs_tot[:], rsum[:, 0:M_TILES],
                                 axis=mybir.AxisListType.X)

            stack3 = ptile([128, 4], F32, name="stack3")
            nc.vector.tensor_copy(stack3[:, 0:1], rs_tot[:])
            nc.vector.tensor_copy(stack3[:, 1:2], tq_acc[:])
            nc.vector.tensor_copy(stack3[:, 2:3], ts_acc[:])
            ps3 = sc_psum.tile([1, 4], F32, tag="ps3")
            nc.tensor.matmul(ps3[:, 0:3], ones_red[:], stack3[:, 0:3],
                             start=True, stop=True)
            sums = ptile([1, 4], F32, name="sums")
            nc.vector.tensor_copy(sums[:, 0:3], ps3[:, 0:3])
            # partial = rs_tot + (N - Ns) * Tq
            part11 = ptile([1, 1], F32, name="part11")
            nc.vector.tensor_scalar(part11[:], sums[:, 1:2], sc["n_minus_ns"],
                                    None, OP.mult)
            part11b = ptile([1, 1], F32, name="part11b")
            nc.vector.tensor_add(part11b[:], part11[:], sums[:, 0:1])

            # ---- AllReduce of the scalar partial ----
            ar_in = dram.tile([1, 1], F32, name="ar_in", tag="ar_in")
            ar_out = dram.tile([1, 1], F32, name="ar_out", tag="ar_out")
            nc.gpsimd.dma_start(ar_in[:], part11b[:])
            nc.gpsimd.collective_compute(
                "AllReduce", OP.add,
                replica_groups=[list(range(N_CORES))],
                ins=[ar_in.opt()], outs=[ar_out.opt()],
            )
            ar_sb = ptile([1, 1], F32, name="ar_sb")
            nc.gpsimd.dma_start(ar_sb[:], ar_out[:])

            # ---- norm and broadcast scalars ----
            # norm = (ar + (N - Nq) * Ts) * inv_n2
            t5 = ptile([1, 1], F32, name="t5")
            nc.vector.tensor_scalar(t5[:], sums[:, 2:3], sc["n_minus_nq"],
                                    None, OP.mult)
            t6 = ptile([1, 1], F32, name="t6")
            nc.vector.tensor_add(t6[:], t5[:], ar_sb[:])
            norm11 = ptile([1, 1], F32, name="norm11")
            nc.vector.tensor_scalar(norm11[:], t6[:], sc["inv_n2"], None, OP.mult)
            # reciprocal + one Newton step:  r1 = r0 * (2 - norm * r0)
            r0 = ptile([1, 1], F32, name="r0")
            nc.vector.reciprocal(r0[:], norm11[:])
            e0 = ptile([1, 1], F32, name="e0")
            nc.vector.tensor_mul(e0[:], norm11[:], r0[:])
            e1 = ptile([1, 1], F32, name="e1")
            nc.vector.tensor_scalar(e1[:], e0[:], -1.0, 2.0, OP.mult, OP.add)
            r1 = ptile([1, 1], F32, name="r1")
            nc.vector.tensor_mul(r1[:], r0[:], e1[:])
            # wn = w / norm
            wn11 = ptile([1, 1], F32, name="wn11")
            nc.vector.tensor_scalar(wn11[:], r1[:], sc["w"], None, OP.mult)

            bc_in = ptile([1, 2], F32, name="bc_in")
            nc.vector.tensor_copy(bc_in[:, 0:1], norm11[:])
            nc.vector.tensor_copy(bc_in[:, 1:2], wn11[:])
            ps_bc = sc_psum.tile([128, 2], F32, tag="bc")
            nc.tensor.matmul(ps_bc[:], ones_bcast[:], bc_in[:, 0:2],
                             start=True, stop=True)
            bc_sb = ptile([128, 2], F32, name="bc_sb")
            nc.vector.tensor_copy(bc_sb[:], ps_bc[:])

            # ---- final per-row normalize:  out = b + (w/norm)*(norm - min(rm, norm)) ----
            for m in range(M_TILES):
                rm = work.tile([128, 1], F32, tag="rm")
                nc.vector.tensor_scalar(rm[:], rsum[:, m:m + 1], sc["inv_count"],
                                        None, OP.mult)
                mn = work.tile([128, 1], F32, tag="mn")
                nc.vector.tensor_tensor(mn[:], rm[:], bc_sb[:, 0:1], op=OP.min)
                df = work.tile([128, 1], F32, tag="df")
                nc.vector.tensor_sub(df[:], bc_sb[:, 0:1], mn[:])
                sc_t = work.tile([128, 1], F32, tag="sc_t")
                nc.vector.tensor_mul(sc_t[:], df[:], bc_sb[:, 1:2])
                ov = work.tile([128, 1], F32, tag="ov")
                nc.vector.tensor_scalar(ov[:], sc_t[:], sc["b"], None, OP.add)
                nc.sync.dma_start(out[m * 128:(m + 1) * 128, :], ov[:])

            # ---- constant output for rows with qvs==0 ----
            cm = tiny.tile([1, 1], F32, tag="cm")
            nc.vector.tensor_scalar(cm[:], sums[:, 2:3], sc["inv_count"],
                                    None, OP.mult)
            cmn = tiny.tile([1, 1], F32, tag="cmn")
            nc.vector.tensor_tensor(cmn[:], cm[:], norm11[:], op=OP.min)
            cdf = tiny.tile([1, 1], F32, tag="cdf")
            nc.vector.tensor_sub(cdf[:], norm11[:], cmn[:])
            csc = tiny.tile([1, 1], F32, tag="csc")
            nc.vector.tensor_mul(csc[:], cdf[:], wn11[:])
            co = tiny.tile([1, 1], F32, tag="co")
            nc.vector.tensor_scalar(co[:], csc[:], sc["b"], None, OP.add)
            nc.sync.dma_start(outc[:, :], co[:])

    nc.compile()
    return nc


def _numpy_fallback(seq, qvs_idx, sum_idx, weight, bias):
    inseqS = seq * sum_idx
    inseqQ = seq * qvs_idx
    q2 = (inseqQ * inseqQ).sum(1)
    s2 = (inseqS * inseqS).sum(1)
    d2 = q2[:, None] + s2[None, :] - 2.0 * (inseqQ @ inseqS.T)
    d2 = np.maximum(d2, 0.0)
    dist = np.sqrt(d2)
    norm = dist.mean()
    colmask = (sum_idx[:, 0] != 0).astype(np.float32)
    count = colmask.sum()
    simcov4 = (dist @ colmask[:, None]) / count
    simcov4 = np.minimum(simcov4, norm)
    simcov4 = 1.0 - simcov4 / norm
    return (simcov4 @ weight + bias).astype(np.float32)


def kernel(seq, qvs_idx, sum_idx, weight, bias):
    seq = np.asarray(seq, dtype=np.float32)
    qvs_idx = np.asarray(qvs_idx, dtype=np.float32)
    sum_idx = np.asarray(sum_idx, dtype=np.float32)
    weight = np.asarray(weight, dtype=np.float32)
    bias = np.asarray(bias, dtype=np.float32)

    N = seq.shape[0]
    qmask = qvs_idx[:, 0] != 0
    smask = sum_idx[:, 0] != 0
    q_idx = np.nonzero(qmask)[0]
    s_idx = np.nonzero(smask)[0]
    Nq, Ns = len(q_idx), len(s_idx)
    if Nq == 0 or Ns == 0:
        return _numpy_fallback(seq, qvs_idx, sum_idx, weight, bias)

    R = max(128, -(-Nq // (N_CORES * 128)) * 128)  # rows per core
    NS_PAD = max(512, -(-Ns // 512) * 512)

    # ---- host-side prep (bf16 rounding matches what the PE will see) ----
    q_bf = seq[q_idx].astype(BF16_NP)                      # [Nq, D]
    s_bf = seq[s_idx].astype(BF16_NP)                      # [Ns, D]
    q2 = (q_bf.astype(np.float32) ** 2).sum(1)             # [Nq]
    s2 = (s_bf.astype(np.float32) ** 2).sum(1)             # [Ns]

    qpad = np.zeros((N_CORES * R, D), dtype=BF16_NP)
    qpad[:Nq] = q_bf * BF16_NP(-2.0)
    q2b_pad = np.full(N_CORES * R, NEG_BIG, dtype=np.float32)
    q2b_pad[:Nq] = q2
    q2f_pad = np.zeros(N_CORES * R, dtype=np.float32)
    q2f_pad[:Nq] = q2

    spad = np.zeros((NS_PAD, D), dtype=BF16_NP)
    spad[:Ns] = s_bf
    st_all = np.ascontiguousarray(spad.T).reshape(K_TILES, 128, NS_PAD)
    s2aug = np.full((1, NS_PAD), NEG_BIG, dtype=np.float32)
    s2aug[0, :Ns] = s2
    s2aug = s2aug.astype(BF16_NP)
    s2f_pad = np.zeros(NS_PAD, dtype=np.float32)
    s2f_pad[:Ns] = s2
    s2f_all = np.ascontiguousarray(s2f_pad.reshape(-1, 128).T)  # [128, NS_PAD/128]

    count = float(Ns)
    scalars = dict(
        inv_count=1.0 / count,
        n_minus_ns=float(N - Ns),
        n_minus_nq=float(N - Nq),
        inv_n2=1.0 / (float(N) * float(N)),
        w=float(weight[0, 0]),
        b=float(bias[0]),
    )

    key = (R, NS_PAD, tuple(sorted(scalars.items())))
    if key not in _cache:
        _cache[key] = _build_program(R, NS_PAD, scalars)
    nc = _cache[key]

    in_maps = []
    for c in range(N_CORES):
        rows = slice(c * R, (c + 1) * R)
        qt_c = np.ascontiguousarray(qpad[rows].T).reshape(K_TILES, 128, R)
        q2b_c = np.ascontiguousarray(q2b_pad[rows].reshape(-1, 128).T)
        q2f_c = np.ascontiguousarray(q2f_pad[rows].reshape(-1, 128).T)
        in_maps.append({
            "qt": qt_c,
            "st": st_all,
            "s2aug": s2aug,
            "q2b": q2b_c,
            "q2f": q2f_c,
            "s2f": s2f_all,
        })

    res = run_bass_kernel_spmd(nc, in_maps, core_ids=list(range(N_CORES)),
                               trace=False)

    vals = np.concatenate([res.results[c]["out"][:, 0] for c in range(N_CORES)])
    full = np.empty(N, dtype=np.float32)
    full[q_idx] = vals[:Nq]
    full[~qmask] = res.results[0]["outc"][0, 0]
    return full.reshape(N, 1)
